# revision 1
# baseline (speedup 1.0000x reference)
"""DialogueGCN forward on 8 Trainium2 NeuronCores (Bass/Tile).

kernel(**inputs) -> np.ndarray [8192, 6] log-probs, matching reference().

Sharding: nodes row-sharded 1024/core. Edges sorted by destination; each core
owns the edges into its dst strip. Graph aggregation = dma_gather of per-edge
feature rows from DRAM + segment-sum as selection matmuls (128-edge blocks
against 32-dst groups). Cross-core: AllGather of h1 and h2. Dense attention is
row-sharded (queries = own strip, keys/values = full graph), computed in fp32
logits + fp16 softmax/PV, streaming keys in two halves.
"""
import numpy as np

import concourse.bass as bass
import concourse.tile as tile
import concourse.mybir as mybir
from concourse import bacc
from concourse.bass_utils import run_bass_kernel_spmd

f32 = mybir.dt.float32
f32r = mybir.dt.float32r
f16 = mybir.dt.float16
i16 = mybir.dt.int16

N, E, F, H, R, NB, NC = 8192, 680000, 200, 100, 8, 30, 6
CORES = 8
NPC = N // CORES            # 1024 dst rows per core
NGRP = NPC // 32            # 32-dst groups per core

AF = mybir.ActivationFunctionType
ALU = mybir.AluOpType
AX = mybir.AxisListType

_ker_cache = {}
_last_res = None


# ------------------------------------------------------------------ host prep
def _wrap_idx(idx):
    """int16 gather-index layout: j -> [j%16, j//16], replicated to 128 rows."""
    n = idx.shape[0]
    t = np.zeros((16, max(n // 16, 1)), np.int16)
    t[np.arange(n) % 16, np.arange(n) // 16] = idx.astype(np.int16)
    return np.tile(t, (8, 1))


def _prep(edge_index, edge_type):
    src = np.asarray(edge_index[0], np.int64)
    dst = np.asarray(edge_index[1], np.int64)
    et = np.asarray(edge_type, np.int64)

    deg = np.bincount(dst * R + et, minlength=N * R).astype(np.float64)
    inv = np.where(deg > 0, 1.0 / np.maximum(deg, 1.0), 0.0).astype(
        np.float32).reshape(N, R)

    core_of = dst // NPC
    grp_of = (dst % NPC) // 32

    def layout(nhalf):
        half = (et >= 4).astype(np.int64) if nhalf == 2 else np.zeros(E, np.int64)
        key = (core_of * NGRP + grp_of) * nhalf + half
        order = np.argsort(key, kind="stable")
        cnt = np.bincount(key, minlength=CORES * NGRP * nhalf)
        blocks = -(-cnt.reshape(CORES, NGRP * nhalf) // 128)
        B = blocks.max(axis=0)                      # static per (grp[,half])
        flat = np.zeros(CORES * NGRP * nhalf + 1, np.int64)
        flat[1:] = np.cumsum(cnt)
        return order, B, flat

    ord1, B1, flat1 = layout(2)
    ord2, B2, flat2 = layout(1)
    totB1, totB2 = int(B1.sum()), int(B2.sum())

    per_core = []
    for c in range(CORES):
        nA = int(B1[0::2].sum()) * 128
        nB = int(B1[1::2].sum()) * 128
        idxA = np.zeros(nA, np.int64)
        idxB = np.zeros(nB, np.int64)
        sel1 = np.zeros((totB1, 128, 32), np.float32)
        offA = offB = boff = 0
        for g in range(NGRP):
            for h in range(2):
                k = (c * NGRP + g) * 2 + h
                e = ord1[flat1[k]:flat1[k + 1]]
                n = e.shape[0]
                nb = int(B1[g * 2 + h])
                gi = src[e] * 4 + (et[e] - 4 * h)
                bi = np.arange(n)
                sel1[boff + bi // 128, bi % 128,
                     (dst[e] % NPC) - g * 32] = inv[dst[e], et[e]]
                if h == 0:
                    idxA[offA:offA + n] = gi
                    offA += nb * 128
                else:
                    idxB[offB:offB + n] = gi
                    offB += nb * 128
                boff += nb
        idx2 = np.zeros(totB2 * 128, np.int64)
        sel2 = np.zeros((totB2, 128, 32), np.float32)
        off = boff = 0
        for g in range(NGRP):
            k = c * NGRP + g
            e = ord2[flat2[k]:flat2[k + 1]]
            n = e.shape[0]
            nb = int(B2[g])
            bi = np.arange(n)
            sel2[boff + bi // 128, bi % 128, (dst[e] % NPC) - g * 32] = 1.0
            idx2[off:off + n] = src[e]
            off += nb * 128
            boff += nb
        per_core.append(dict(
            idxA=_wrap_idx(idxA), idxB=_wrap_idx(idxB), idx2=_wrap_idx(idx2),
            sel1=np.ascontiguousarray(sel1.transpose(1, 0, 2)).reshape(128, -1),
            sel2=np.ascontiguousarray(sel2.transpose(1, 0, 2)).reshape(128, -1)))

    meta = dict(B1=[(int(B1[g * 2]), int(B1[g * 2 + 1])) for g in range(NGRP)],
                B2=[int(b) for b in B2], totB1=totB1, totB2=totB2,
                lenA=per_core[0]["idxA"].shape[1] * 16,
                lenB=per_core[0]["idxB"].shape[1] * 16)
    return per_core, meta


# ------------------------------------------------------------------ program
def _build(meta, phase="full"):
    B1, B2 = meta["B1"], meta["B2"]
    totB1, totB2 = meta["totB1"], meta["totB2"]
    LA, LB = meta["lenA"], meta["lenB"]
    L2 = totB2 * 128
    MB1 = max(max(b) for b in B1)
    MB2 = max(B2)
    KH = N // 2                 # keys per attention half-pass

    nc = bacc.Bacc("TRN2", target_bir_lowering=False, debug=False,
                   num_devices=CORES)
    P = lambda n, s, d: nc.declare_dram_parameter(n, s, d, isOutput=False)

    xTd = P("xT", [F, N], f32)
    xd = P("x", [N, F], f32)
    basisd = P("basis", [NB, F * H], f32)
    compTd = P("compT", [NB, R], f32)
    rootwd = P("root_w", [F, H], f32)
    rootbd = P("root_b", [1, H], f32)
    gcreld = P("gc_rel_w", [H, H], f32)
    gcrelbd = P("gc_rel_b", [1, H], f32)
    gcrootd = P("gc_root_w", [H, H], f32)
    betawd = P("beta_w", [3 * H, 3 * H], f32)
    betabd = P("beta_b", [1, 3 * H], f32)
    linw16d = P("lin_w16", [3 * H, H], f16)
    linbd = P("lin_b", [1, H], f32)
    smaxw16d = P("smax_w16", [H, NC], f16)
    smaxbd = P("smax_b", [1, NC], f32)
    id32d = P("ident32", [128, 128], f32)
    id16d = P("ident16", [128, 128], f16)
    onesd = P("ones", [1, 512], f32)
    xTsd = P("xTs", [F, NPC], f32)
    idxAd = P("idxA", [128, LA // 16], i16)
    idxBd = P("idxB", [128, LB // 16], i16)
    idx2d = P("idx2", [128, L2 // 16], i16)
    sel1d = P("sel1", [128, totB1 * 32], f32)
    sel2d = P("sel2", [128, totB2 * 32], f32)

    outd = nc.declare_dram_parameter("out", [NPC, NC], f32, isOutput=True)
    dbgd = None
    if phase != "full":
        dbgd = nc.declare_dram_parameter("dbg", [N, 128], f32, isOutput=True)

    with tile.TileContext(nc, num_cores=CORES) as tc:
        with tc.tile_pool(name="dram", bufs=1, space="DRAM") as dram, \
             tc.tile_pool(name="persist", bufs=1) as pp:

            xwA = dram.tile([N * 4, 128], f32, tag="xwA")
            xwB = dram.tile([N * 4, 128], f32, tag="xwB")
            wtmp_d = dram.tile([R, F, H], f32, tag="wtmp")
            h1s_d = dram.tile([NPC, 128], f32, tag="h1s")
            h1f_d = dram.tile([N, 128], f32, tag="h1f")
            h2s_d = dram.tile([NPC, 128], f32, tag="h2s")
            h2f_d = dram.tile([N, 128], f32, tag="h2f")
            h2T_d = dram.tile([H, N], f32, tag="h2T_d")

            # --------- persistent small SBUF ---------
            xTs = pp.tile([100, 2, NPC], f32, tag="xTs")
            nc.sync.dma_start(xTs[:, 0, :], xTsd[0:100, :])
            nc.sync.dma_start(xTs[:, 1, :], xTsd[100:200, :])
            rootw = pp.tile([100, 2, H], f32, tag="rootw")
            nc.sync.dma_start(rootw[:, 0, :], rootwd[0:100, :])
            nc.sync.dma_start(rootw[:, 1, :], rootwd[100:200, :])
            rootb = pp.tile([1, H], f32, tag="rootb")
            nc.sync.dma_start(rootb[:], rootbd[:])
            gcrel = pp.tile([H, H], f32, tag="gcrel")
            nc.sync.dma_start(gcrel[:], gcreld[:])
            gcrelb = pp.tile([1, H], f32, tag="gcrelb")
            nc.sync.dma_start(gcrelb[:], gcrelbd[:])
            gcroot = pp.tile([H, H], f32, tag="gcroot")
            nc.sync.dma_start(gcroot[:], gcrootd[:])
            betaw = pp.tile([100, 3, 3, 100], f32, tag="betaw")
            nc.sync.dma_start(
                betaw[:], betawd[:].rearrange("(fc f) (gc g) -> f fc gc g",
                                              fc=3, gc=3))
            betab = pp.tile([1, 3 * H], f32, tag="betab")
            nc.sync.dma_start(betab[:], betabd[:])
            linw = pp.tile([100, 3, H], f16, tag="linw")
            nc.sync.dma_start(linw[:], linw16d[:].rearrange("(gc g) j -> g gc j",
                                                            gc=3))
            linb = pp.tile([1, H], f32, tag="linb")
            nc.sync.dma_start(linb[:], linbd[:])
            smaxw = pp.tile([H, NC], f16, tag="smaxw")
            nc.sync.dma_start(smaxw[:], smaxw16d[:])
            smaxb = pp.tile([1, NC], f32, tag="smaxb")
            nc.sync.dma_start(smaxb[:], smaxbd[:])
            id32 = pp.tile([128, 128], f32, tag="id32")
            nc.sync.dma_start(id32[:], id32d[:])
            id16 = pp.tile([128, 128], f16, tag="id16")
            nc.sync.dma_start(id16[:], id16d[:])
            ones = pp.tile([1, 512], f32, tag="ones")
            nc.sync.dma_start(ones[:], onesd[:])
            h1T = pp.tile([128, NPC], f32, tag="h1T")
            h2T = pp.tile([128, NPC], f32, tag="h2T")
            nc.vector.memset(h2T[:], 0.0)

            # ================= phase A: W then xW -> xwA/xwB =================
            with tc.tile_pool(name="pa", bufs=2) as pa, \
                 tc.tile_pool(name="pa1", bufs=1) as pa1, \
                 tc.tile_pool(name="psa", bufs=2, space="PSUM") as psa:
                basis_t = pa1.tile([NB, F * H], f32, tag="basis")
                nc.sync.dma_start(basis_t[:], basisd[:])
                compT = pa1.tile([NB, R], f32, tag="compT")
                nc.sync.dma_start(compT[:], compTd[:])
                wtmp_flat = wtmp_d[:].rearrange("r f h -> r (f h)")
                for t in range(F * H // 500):
                    pw = psa.tile([R, 500], f32, tag="pw")
                    nc.tensor.matmul(pw[:], compT[:],
                                     basis_t[:, t * 500:(t + 1) * 500],
                                     start=True, stop=True)
                    wb = pa.tile([R, 500], f32, tag="wb")
                    nc.vector.tensor_copy(wb[:], pw[:])
                    nc.sync.dma_start(wtmp_flat[:, t * 500:(t + 1) * 500], wb[:])
                wrhs = pa1.tile([100, 2, R * H], f32r, tag="wrhs")
                wld = pa1.tile([100, 2, R * H], f32, tag="wld")
                for c in range(2):
                    nc.sync.dma_start(
                        wld[:, c, :].rearrange("f (r h) -> f r h", h=H),
                        wtmp_d[:, c * 100:(c + 1) * 100, :].rearrange(
                            "r f h -> f r h"))
                nc.vector.tensor_copy(wrhs[:], wld[:])

                for nt in range(N // 128):
                    nsl = slice(nt * 128, (nt + 1) * 128)
                    xtl = pa.tile([100, 2, 128], f32, tag="xtl")
                    nc.sync.dma_start(xtl[:, 0, :], xTd[0:100, nsl])
                    nc.sync.dma_start(xtl[:, 1, :], xTd[100:200, nsl])
                    xtr = pa.tile([100, 2, 128], f32r, tag="xtr")
                    nc.vector.tensor_copy(xtr[:], xtl[:])
                    pxw = psa.tile([128, 2, 512], f32, tag="pxw")
                    for c in range(2):
                        for hf in range(2):
                            nc.tensor.matmul(pxw[:, hf, 0:400], xtr[:, c, :],
                                             wrhs[:, c,
                                                  hf * 400:(hf + 1) * 400],
                                             start=(c == 0), stop=(c == 1))
                    stage = pa.tile([128, R, 128], f32, tag="stage")
                    nc.vector.memset(stage[:, :, 100:128], 0.0)
                    for hf in range(2):
                        nc.vector.tensor_copy(
                            stage[:, hf * 4:(hf + 1) * 4, 0:100],
                            pxw[:, hf, 0:400].rearrange("p (r h) -> p r h",
                                                        h=H))
                    nc.sync.dma_start(
                        xwA[nt * 512:(nt + 1) * 512, :].rearrange(
                            "(p r) e -> p r e", r=4), stage[:, 0:4, :])
                    nc.sync.dma_start(
                        xwB[nt * 512:(nt + 1) * 512, :].rearrange(
                            "(p r) e -> p r e", r=4), stage[:, 4:8, :])

            if phase == "A":
                nc.sync.dma_start(
                    dbgd[:].rearrange("(b p) e -> p b e", p=128),
                    xwA[0:N, :].rearrange("(b p) e -> p b e", p=128))
            # ================= phases B/C: segment aggregation ===============
            def seg_stage(s):
                sel_d = sel1d if s == 1 else sel2d
                outT = h1T if s == 1 else h2T
                str_d = h1s_d if s == 1 else h2s_d
                full_d = h1f_d if s == 1 else h2f_d
                MB = MB1 if s == 1 else MB2
                with tc.tile_pool(name=f"pb{s}", bufs=3) as pb, \
                     tc.tile_pool(name=f"pq{s}", bufs=2) as pq, \
                     tc.tile_pool(name=f"pi{s}", bufs=1) as pi, \
                     tc.tile_pool(name=f"psb{s}", bufs=2, space="PSUM") as psb:
                    if s == 1:
                        idxA_t = pi.tile([128, LA // 16], i16, tag="idxA")
                        nc.sync.dma_start(idxA_t[:], idxAd[:])
                        idxB_t = pi.tile([128, LB // 16], i16, tag="idxB")
                        nc.sync.dma_start(idxB_t[:], idxBd[:])
                    else:
                        idx2_t = pi.tile([128, L2 // 16], i16, tag="idx2")
                        nc.sync.dma_start(idx2_t[:], idx2d[:])
                    GB = 8      # blocks per dma_gather (1024-idx HW limit)
                    boff = 0
                    offA = offB = off2 = 0
                    for ch in range(NPC // 128):
                        seq = []
                        for gg in range(4):
                            g = ch * 4 + gg
                            if s == 1:
                                seq += [(gg, 0, B1[g][0]), (gg, 1, B1[g][1])]
                            else:
                                seq.append((gg, 0, B2[g]))
                        seq = [t for t in seq if t[2] > 0]
                        totmm = sum(nb for _, _, nb in seq)
                        mmi = 0
                        ph = psb.tile([128, 128], f32, tag="ph")
                        for gg, h, nb in seq:
                            done = 0
                            while done < nb:
                                k = min(GB, nb - done)
                                mg = pb.tile([128, GB, 128], f32, tag="mg")
                                if s == 1 and h == 0:
                                    nc.gpsimd.dma_gather(
                                        mg[:, 0:k, :], xwA[:],
                                        idxA_t[:, offA // 16:
                                               (offA + k * 128) // 16],
                                        num_idxs=k * 128, num_idxs_reg=k * 128,
                                        elem_size=128)
                                    offA += k * 128
                                elif s == 1:
                                    nc.gpsimd.dma_gather(
                                        mg[:, 0:k, :], xwB[:],
                                        idxB_t[:, offB // 16:
                                               (offB + k * 128) // 16],
                                        num_idxs=k * 128, num_idxs_reg=k * 128,
                                        elem_size=128)
                                    offB += k * 128
                                else:
                                    nc.gpsimd.dma_gather(
                                        mg[:, 0:k, :], h1f_d[:],
                                        idx2_t[:, off2 // 16:
                                               (off2 + k * 128) // 16],
                                        num_idxs=k * 128, num_idxs_reg=k * 128,
                                        elem_size=128)
                                    off2 += k * 128
                                sel_t = pb.tile([128, GB, 32], f32, tag="sel")
                                nc.sync.dma_start(
                                    sel_t[:, 0:k, :],
                                    sel_d[:, boff * 32:(boff + k) * 32]
                                    .rearrange("p (b c) -> p b c", c=32))
                                for b in range(k):
                                    nc.tensor.matmul(
                                        ph[:, gg * 32:(gg + 1) * 32],
                                        mg[:, b, :], sel_t[:, b, :],
                                        start=(mmi == 0),
                                        stop=(s == 2 and mmi == totmm - 1))
                                    mmi += 1
                                boff += k
                                done += k
                        dsl = slice(ch * 128, (ch + 1) * 128)
                        if s == 1:
                            for c in range(2):
                                nc.tensor.matmul(ph[0:H, :], rootw[:, c, :],
                                                 xTs[:, c, dsl],
                                                 start=False, stop=False)
                            nc.tensor.matmul(ph[0:H, :], rootb[:],
                                             ones[:, 0:128], start=False,
                                             stop=True)
                            nc.vector.tensor_copy(outT[:, dsl], ph[:])
                        else:
                            a2 = pq.tile([128, 128], f32, tag="a2")
                            nc.vector.tensor_copy(a2[:], ph[:])
                            p2 = psb.tile([128, 128], f32, tag="p2")
                            nc.tensor.matmul(p2[0:H, :], gcrel[:], a2[0:H, :],
                                             start=True, stop=False)
                            nc.tensor.matmul(p2[0:H, :], gcroot[:],
                                             h1T[0:H, dsl], start=False,
                                             stop=False)
                            nc.tensor.matmul(p2[0:H, :], gcrelb[:],
                                             ones[:, 0:128], start=False,
                                             stop=True)
                            nc.vector.tensor_copy(outT[0:H, dsl], p2[0:H, :])
                        ptr = psb.tile([128, 128], f32, tag="ptr")
                        nc.tensor.matmul(ptr[:], outT[:, dsl], id32[:],
                                         is_transpose=True, start=True,
                                         stop=True)
                        nodem = pq.tile([128, 128], f32, tag="nodem")
                        nc.vector.tensor_copy(nodem[:], ptr[:])
                        nc.sync.dma_start(str_d[dsl, :], nodem[:])
                nc.gpsimd.collective_compute(
                    "AllGather", ALU.bypass,
                    replica_groups=[list(range(CORES))],
                    ins=[str_d[:].opt()], outs=[full_d[:].opt()])

            if phase != "A":
                seg_stage(1)
            if phase == "B":
                nc.sync.dma_start(
                    dbgd[:].rearrange("(b p) e -> p b e", p=128),
                    h1f_d[:].rearrange("(b p) e -> p b e", p=128))
            if phase in ("C", "full"):
                seg_stage(2)
            if phase == "C":
                nc.sync.dma_start(
                    dbgd[:].rearrange("(b p) e -> p b e", p=128),
                    h2f_d[:].rearrange("(b p) e -> p b e", p=128))

            # ================= phase D: attention =================
            if phase == "full":
                # h2f -> h2T_d (feature-major, DRAM)
                with tc.tile_pool(name="pt0", bufs=3) as pt0, \
                     tc.tile_pool(name="pst0", bufs=2, space="PSUM") as pst0:
                    for kb in range(N // 128):
                        blk = pt0.tile([128, 128], f32, tag="blk")
                        nc.sync.dma_start(blk[:], h2f_d[kb * 128:(kb + 1) * 128, :])
                        pt = pst0.tile([128, 128], f32, tag="pt")
                        nc.tensor.matmul(pt[:], blk[:], id32[:], is_transpose=True,
                                         start=True, stop=True)
                        h2tb = pt0.tile([100, 128], f32, tag="h2tb")
                        nc.vector.tensor_copy(h2tb[:], pt[0:100, :])
                        nc.sync.dma_start(h2T_d[:, kb * 128:(kb + 1) * 128], h2tb[:])

                # beforeT [100, 3(gc), NPC] from strip emoT
                befT = pp.tile([100, 3, NPC], f32, tag="befT")
                with tc.tile_pool(name="psf", bufs=2, space="PSUM") as psf:
                    emoTs = (xTs[:, 0, :], xTs[:, 1, :], h2T[0:H, :])
                    for gc in range(3):
                        for qh in range(NPC // 512):
                            qsl = slice(qh * 512, qh * 512 + 512)
                            pb_ = psf.tile([100, 512], f32, tag="pbef")
                            for fc in range(3):
                                nc.tensor.matmul(pb_[:], betaw[:, fc, gc, :],
                                                 emoTs[fc][:, qsl],
                                                 start=(fc == 0), stop=False)
                            nc.tensor.matmul(pb_[:],
                                             betab[:, gc * 100:(gc + 1) * 100],
                                             ones[:, 0:512], start=False, stop=True)
                            nc.vector.tensor_copy(befT[:, gc, qsl], pb_[:])

                # two half-passes over keys; flash combine
                NQT = NPC // 128
                m_st = pp.tile([128, 2, NQT], f32, tag="m_st")
                s_st = pp.tile([128, 2, NQT], f32, tag="s_st")
                em_st = pp.tile([128, 2, NQT, 3 * H], f32, tag="em_st")
                for kh in range(2):
                    with tc.tile_pool(name=f"pk{kh}", bufs=1) as pk, \
                         tc.tile_pool(name=f"pl{kh}", bufs=2) as pl, \
                         tc.tile_pool(name=f"psk{kh}", bufs=2, space="PSUM") as psk:
                        ksl_d = slice(kh * KH, (kh + 1) * KH)
                        keys = pk.tile([100, 3, KH], f32, tag="keys")
                        nc.sync.dma_start(keys[:, 0, :], xTd[0:100, ksl_d])
                        nc.sync.dma_start(keys[:, 1, :], xTd[100:200, ksl_d])
                        nc.sync.dma_start(keys[:, 2, :], h2T_d[:, ksl_d])
                        emoV = pk.tile([128, KH // 128, 3 * H], f16, tag="emoV")
                        nc.gpsimd.dma_start(
                            emoV[:, :, 0:F],
                            xd[ksl_d, :].rearrange("(kb p) f -> p kb f", p=128))
                        nc.gpsimd.dma_start(
                            emoV[:, :, F:F + H],
                            h2f_d[ksl_d, 0:H].rearrange("(kb p) f -> p kb f", p=128))
                        Srow = pk.tile([128, KH], f32, tag="Srow")
                        Prow = pk.tile([128, KH], f16, tag="Prow")
                        for qt in range(NQT):
                            qsl = slice(qt * 128, (qt + 1) * 128)
                            for kt in range(KH // 512):
                                ksl = slice(kt * 512, (kt + 1) * 512)
                                psS = psk.tile([128, 512], f32, tag="psS")
                                for fc in range(3):
                                    nc.tensor.matmul(psS[:], befT[:, fc, qsl],
                                                     keys[:, fc, ksl],
                                                     start=(fc == 0), stop=(fc == 2))
                                nc.vector.tensor_copy(Srow[:, ksl], psS[:])
                            mx = pl.tile([128, 1], f32, tag="mx")
                            nc.vector.reduce_max(mx[:], Srow[:], axis=AX.XYZW)
                            nc.vector.tensor_copy(m_st[:, kh, qt:qt + 1], mx[:])
                            nmx = pl.tile([128, 1], f32, tag="nmx")
                            nc.vector.tensor_scalar_mul(nmx[:], mx[:], -1.0)
                            ssum = pl.tile([128, 1], f32, tag="ssum")
                            nc.scalar.activation(Prow[:], Srow[:], AF.Exp,
                                                 bias=nmx[:], scale=1.0,
                                                 accum_out=ssum[:])
                            nc.vector.tensor_copy(s_st[:, kh, qt:qt + 1], ssum[:])
                            pem = psk.tile([128, 3 * H], f32, tag="pem")
                            nkb = KH // 128
                            for kb in range(nkb):
                                ptp = psk.tile([128, 128], f16, tag="ptp")
                                nc.tensor.matmul(ptp[:],
                                                 Prow[:, kb * 128:(kb + 1) * 128],
                                                 id16[:], is_transpose=True,
                                                 start=True, stop=True)
                                pts = pl.tile([128, 128], f16, tag="pts")
                                nc.vector.tensor_copy(pts[:], ptp[:])
                                nc.tensor.matmul(pem[:], pts[:], emoV[:, kb, :],
                                                 start=(kb == 0),
                                                 stop=(kb == nkb - 1))
                            nc.vector.tensor_copy(em_st[:, kh, qt, :], pem[:])

                # combine halves + head
                with tc.tile_pool(name="ph2", bufs=2) as ph2, \
                     tc.tile_pool(name="psh", bufs=2, space="PSUM") as psh:
                    for qt in range(NQT):
                        mm_ = ph2.tile([128, 1], f32, tag="mm_")
                        nc.vector.tensor_tensor(mm_[:], m_st[:, 0, qt:qt + 1],
                                                m_st[:, 1, qt:qt + 1], op=ALU.max)
                        al = ph2.tile([128, 2], f32, tag="al")
                        d0 = ph2.tile([128, 2], f32, tag="d0")
                        nc.vector.tensor_scalar(d0[:], m_st[:, :, qt], mm_[:], None,
                                                op0=ALU.subtract)
                        nc.scalar.activation(al[:], d0[:], AF.Exp)
                        sw = ph2.tile([128, 2], f32, tag="sw")
                        nc.vector.tensor_tensor(sw[:], s_st[:, :, qt], al[:],
                                                op=ALU.mult)
                        den = ph2.tile([128, 1], f32, tag="den")
                        nc.vector.reduce_sum(den[:], sw[:], axis=AX.XYZW)
                        rcp = ph2.tile([128, 1], f32, tag="rcp")
                        nc.vector.reciprocal(rcp[:], den[:])
                        e0 = ph2.tile([128, 3 * H], f32, tag="e0")
                        nc.vector.tensor_scalar(e0[:], em_st[:, 0, qt, :],
                                                al[:, 0:1], None, op0=ALU.mult)
                        e1 = ph2.tile([128, 3 * H], f32, tag="e1")
                        nc.vector.tensor_scalar(e1[:], em_st[:, 1, qt, :],
                                                al[:, 1:2], None, op0=ALU.mult)
                        es = ph2.tile([128, 3 * H], f32, tag="es")
                        nc.vector.tensor_tensor(es[:], e0[:], e1[:], op=ALU.add)
                        em2 = ph2.tile([128, 3 * H], f16, tag="em2")
                        nc.vector.tensor_scalar(em2[:], es[:], rcp[:], None,
                                                op0=ALU.mult)
                        # head: hiddenT = relu(lin_w.T @ em2.T + lin_b)
                        ph_ = psh.tile([100, 128], f32, tag="phid")
                        for gc in range(3):
                            pe2 = psh.tile([100, 128], f16, tag="pe2t")
                            nc.tensor.matmul(pe2[:],
                                             em2[:, gc * 100:(gc + 1) * 100],
                                             id16[:], is_transpose=True,
                                             start=True, stop=True)
                            e2t = ph2.tile([100, 128], f16, tag="e2t")
                            nc.vector.tensor_copy(e2t[:], pe2[:])
                            nc.tensor.matmul(ph_[:], linw[:, gc, :], e2t[:],
                                             start=(gc == 0), stop=False)
                        nc.tensor.matmul(ph_[:], linb[:], ones[:, 0:128],
                                         start=False, stop=True)
                        hidT = ph2.tile([100, 128], f16, tag="hidT")
                        nc.scalar.activation(hidT[:], ph_[:], AF.Relu)
                        plg = psh.tile([NC, 128], f32, tag="plg")
                        nc.tensor.matmul(plg[:], smaxw[:], hidT[:], start=True,
                                         stop=False)
                        nc.tensor.matmul(plg[:], smaxb[:], ones[:, 0:128],
                                         start=False, stop=True)
                        lgT = ph2.tile([NC, 128], f32, tag="lgT")
                        nc.vector.tensor_copy(lgT[:], plg[:])
                        plt = psh.tile([128, NC], f32, tag="plt")
                        nc.tensor.matmul(plt[:], lgT[:], id32[0:NC, 0:NC],
                                         is_transpose=True, start=True, stop=True)
                        lg = ph2.tile([128, NC], f32, tag="lg")
                        nc.vector.tensor_copy(lg[:], plt[:])
                        m6 = ph2.tile([128, 1], f32, tag="m6")
                        nc.vector.reduce_max(m6[:], lg[:], axis=AX.XYZW)
                        nm6 = ph2.tile([128, 1], f32, tag="nm6")
                        nc.vector.tensor_scalar_mul(nm6[:], m6[:], -1.0)
                        e6 = ph2.tile([128, NC], f32, tag="e6")
                        s6 = ph2.tile([128, 1], f32, tag="s6")
                        nc.scalar.activation(e6[:], lg[:], AF.Exp, bias=nm6[:],
                                             scale=1.0, accum_out=s6[:])
                        ls6 = ph2.tile([128, 1], f32, tag="ls6")
                        nc.scalar.activation(ls6[:], s6[:], AF.Ln)
                        sh = ph2.tile([128, 1], f32, tag="sh")
                        nc.vector.tensor_add(sh[:], m6[:], ls6[:])
                        outt = ph2.tile([128, NC], f32, tag="outt")
                        nc.vector.tensor_scalar(outt[:], lg[:], sh[:], None,
                                                op0=ALU.subtract)
                        nc.sync.dma_start(outd[qt * 128:(qt + 1) * 128, :], outt[:])

    nc.compile()
    return nc


# ------------------------------------------------------------------ entry
def kernel(x, edge_index, edge_norm, edge_type, basis, comp, root_w, root_b,
           gc_rel_w, gc_rel_b, gc_root_w, beta_w, beta_b, lin_w, lin_b,
           smax_w, smax_b):
    x = np.ascontiguousarray(np.asarray(x, np.float32))
    per_core, meta = _prep(edge_index, edge_type)

    import os
    phase = os.environ.get("KPHASE", "full")
    key = (phase, meta["totB1"], meta["totB2"], meta["lenA"], meta["lenB"],
           tuple(map(tuple, meta["B1"])), tuple(meta["B2"]))
    if key not in _ker_cache:
        _ker_cache[key] = _build(meta, phase)
    nc = _ker_cache[key]

    shared = dict(
        xT=np.ascontiguousarray(x.T),
        x=x,
        basis=np.ascontiguousarray(
            np.asarray(basis, np.float32).reshape(NB, F * H)),
        compT=np.ascontiguousarray(np.asarray(comp, np.float32).T),
        root_w=np.asarray(root_w, np.float32),
        root_b=np.asarray(root_b, np.float32).reshape(1, H),
        gc_rel_w=np.asarray(gc_rel_w, np.float32),
        gc_rel_b=np.asarray(gc_rel_b, np.float32).reshape(1, H),
        gc_root_w=np.asarray(gc_root_w, np.float32),
        beta_w=np.asarray(beta_w, np.float32),
        beta_b=np.asarray(beta_b, np.float32).reshape(1, 3 * H),
        lin_w16=np.asarray(lin_w, np.float16),
        lin_b=np.asarray(lin_b, np.float32).reshape(1, H),
        smax_w16=np.asarray(smax_w, np.float16),
        smax_b=np.asarray(smax_b, np.float32).reshape(1, NC),
        ident32=np.eye(128, dtype=np.float32),
        ident16=np.eye(128, dtype=np.float16),
        ones=np.ones((1, 512), np.float32),
    )
    in_maps = []
    for c in range(CORES):
        m = dict(shared)
        m["xTs"] = np.ascontiguousarray(x[c * NPC:(c + 1) * NPC, :].T)
        m.update(per_core[c])
        in_maps.append(m)

    res = run_bass_kernel_spmd(nc, in_maps, core_ids=list(range(CORES)),
                               trace_cores=[0])
    global _last_res
    _last_res = res
    if phase != "full":
        return [res.results[c]["dbg"] for c in range(CORES)]
    return np.concatenate([res.results[c]["out"] for c in range(CORES)], axis=0)



# revision 20
# speedup vs baseline: 3.2916x; 3.2916x over previous
"""DialogueGCN forward on 8 Trainium2 NeuronCores (Bass/Tile).

kernel(**inputs) -> np.ndarray [8192, 6] log-probs, matching reference().

Sharding: nodes row-sharded 1024/core; each core owns edges into its dst
strip. No device gathers: stage-1 edge features are host-permuted x rows
(xg, fp16) streamed contiguously and segment-summed into (dst,rel)
buckets via narrow selection matmuls, then transformed with W (linearity
of RGCN). Stage-2 reads the AllGathered h1 contiguously and aggregates
with a dense count matrix streamed as the moving matmul operand.

Precision: the attention logits span +-600, so absolute logit error must
stay ~1e-2 — everything feeding S (W, h1, h2, before, beta_w/gc/root
weights) is kept at f32/f32r; h1 crosses cores as an fp16 hi+lo pair
(matmul dtype pairing), h2 crosses in f32. Keys x-part, V, P, em2 and
the head run in fp16. Softmax uses a true per-row max (S rows in SBUF).
"""
import numpy as np

import concourse.bass as bass
import concourse.tile as tile
import concourse.mybir as mybir
from concourse import bacc
from concourse.bass_utils import run_bass_kernel_spmd

f32 = mybir.dt.float32
f32r = mybir.dt.float32r
f16 = mybir.dt.float16

N, E, F, H, R, NB, NC = 8192, 680000, 200, 100, 8, 30, 6
CORES = 8
NPC = N // CORES            # 1024 dst rows per core
NG = NPC // 32              # 32-dst groups per core (32)
NBIN = NG * R               # bins per core (256)
KB = N // 128               # key blocks (64)

AF = mybir.ActivationFunctionType
ALU = mybir.AluOpType
AX = mybir.AxisListType

_ker_cache = {}
_last_res = None


# ------------------------------------------------------------------ host prep
def _prep(x, edge_index, edge_type):
    src = np.asarray(edge_index[0], np.int64)
    dst = np.asarray(edge_index[1], np.int64)
    et = np.asarray(edge_type, np.int64)

    deg = np.bincount(dst * R + et, minlength=N * R).astype(np.float64)
    inv = np.where(deg > 0, 1.0 / np.maximum(deg, 1.0), 0.0).astype(np.float32)
    invv = inv[dst * R + et]                       # per-edge weight

    core = dst >> 10
    g = (dst & 1023) >> 5
    binid = ((core * NG + g) << 3) | et            # [0, CORES*NBIN)
    order = np.argsort(binid, kind="stable")
    cnt = np.bincount(binid, minlength=CORES * NBIN).reshape(CORES, NBIN)
    B1 = (-(-cnt // 128)).max(axis=0)              # blocks per bin (static)
    totB1 = int(B1.sum())
    bstart = np.concatenate([[0], np.cumsum(B1)])  # block offset per bin

    flat_cnt = cnt.reshape(-1)
    starts = np.concatenate([[0], np.cumsum(flat_cnt)])
    pos = np.arange(E, dtype=np.int64) - np.repeat(starts[:-1], flat_cnt)
    sbin = binid[order]
    blk_all = bstart[sbin % NBIN] + (pos >> 7)
    row_all = pos & 127

    x16 = np.asarray(x, np.float32).astype(np.float16)
    per_core = []
    for c in range(CORES):
        sl = slice(starts[c * NBIN], starts[(c + 1) * NBIN])
        e = order[sl]
        bl = blk_all[sl]
        rw = row_all[sl]
        xg = np.zeros((128, totB1, F), np.float16)
        xg[rw, bl] = x16[src[e]]
        sel1 = np.zeros((128, totB1, 32), np.float16)
        sel1[rw, bl, dst[e] & 31] = invv[e]
        cnt2 = np.bincount(src[e] * NPC + (dst[e] & 1023), minlength=N * NPC)
        sel2 = np.ascontiguousarray(
            cnt2.reshape(KB, 128, NPC).transpose(1, 0, 2)).astype(np.float16)
        per_core.append(dict(
            xg=xg.reshape(128, totB1 * F),
            sel1=sel1.reshape(128, totB1 * 32),
            sel2=sel2.reshape(128, KB * NPC)))

    meta = dict(B1=[int(b) for b in B1], totB1=totB1)
    return per_core, meta


# ------------------------------------------------------------------ program
def _build(meta, phase="full"):
    B1 = meta["B1"]
    totB1 = meta["totB1"]

    nc = bacc.Bacc("TRN2", target_bir_lowering=False, debug=False,
                   num_devices=CORES)
    P = lambda n, s, d: nc.declare_dram_parameter(n, s, d, isOutput=False)

    xT16d = P("xT16", [F, N], f16)             # keys x-part
    xvd = P("xv", [128, KB * F], f16)          # emoV x-part, [p, kb, f]
    xTs32d = P("xTs32", [100, 2 * NPC], f32)   # own strip xT, f32
    basis32d = P("basis32", [NB, F * H], f32)
    compT32d = P("compT32", [NB, R], f32)
    rootw32d = P("rootw32", [100, 2 * H], f32)
    rootb32d = P("rootb32", [1, H], f32)
    gcrel32d = P("gcrel32", [H, H], f32)
    gcrelb32d = P("gcrelb32", [1, H], f32)
    gcroot32d = P("gcroot32", [H, H], f32)
    betaw32d = P("betaw32", [100, 3 * 3 * H], f32)  # [f, fc, gc, h]
    betab32d = P("betab32", [1, 3 * H], f32)
    linwd = P("linw16", [100, 3 * H], f16)     # chunks [f, c, h]
    linbd = P("linb16", [1, H], f16)
    smaxwd = P("smaxw16", [H, NC], f16)
    smaxbd = P("smaxb16", [1, NC], f16)
    id16d = P("ident16", [128, 128], f16)
    id32d = P("ident32", [128, 128], f32)
    ones32d = P("ones32", [1, 512], f32)
    ones16d = P("ones16", [1, NPC], f16)
    xgd = P("xg", [128, totB1 * F], f16)
    sel1d = P("sel1", [128, totB1 * 32], f16)
    sel2d = P("sel2", [128, KB * NPC], f16)

    outd = nc.declare_dram_parameter("out", [NPC, NC], f32, isOutput=True)
    dbgd = None
    if phase != "full":
        dbgd = nc.declare_dram_parameter("dbg", [N, 2 * H], f16, isOutput=True)

    with tile.TileContext(nc, num_cores=CORES) as tc:
        with tc.tile_pool(name="dram", bufs=1, space="DRAM") as dram, \
             tc.tile_pool(name="persist", bufs=1) as pp:

            wtmp_d = dram.tile([R, F * H], f32, tag="wtmp")
            h1s_d = dram.tile([NPC, 2 * H], f16, tag="h1s")
            h1f_d = dram.tile([N, 2 * H], f16, tag="h1f")
            h2s_d = dram.tile([NPC, H], f32, tag="h2s")
            h2f_d = dram.tile([N, H], f32, tag="h2f")

            # --------- persistent SBUF (small) ---------
            id16 = pp.tile([128, 128], f16, tag="id16")
            nc.sync.dma_start(id16[:], id16d[:])
            id32 = pp.tile([128, 128], f32, tag="id32")
            nc.sync.dma_start(id32[:], id32d[:])
            ones16 = pp.tile([1, NPC], f16, tag="ones16")
            nc.sync.dma_start(ones16[:], ones16d[:])
            linw = pp.tile([100, 3, H], f16, tag="linw")
            nc.sync.dma_start(linw[:], linwd[:].rearrange(
                "p (c h) -> p c h", c=3))
            linb = pp.tile([1, H], f16, tag="linb")
            nc.sync.dma_start(linb[:], linbd[:])
            smaxw = pp.tile([H, NC], f16, tag="smaxw")
            nc.sync.dma_start(smaxw[:], smaxwd[:])
            smaxb = pp.tile([1, NC], f16, tag="smaxb")
            nc.sync.dma_start(smaxb[:], smaxbd[:])

            # f32 weights for the S-precision chain (freed before
            # attention: scoped in pool pm spanning phase W .. bef).
            # Plain fp32 matmuls (4 cyc/row) — f32r measured only ~13-bit.
            pm = tc.tile_pool(name="pm", bufs=1)
            pmp = pm.__enter__()
            onesr = pmp.tile([1, 512], f32, tag="onesr")
            nc.sync.dma_start(onesr[:], ones32d[:])
            rootwr = pmp.tile([100, 2, H], f32, tag="rootwr")
            nc.sync.dma_start(rootwr[:], rootw32d[:].rearrange(
                "p (c h) -> p c h", c=2))
            rootbr = pmp.tile([1, H], f32, tag="rootbr")
            nc.sync.dma_start(rootbr[:], rootb32d[:])
            gcrelr = pmp.tile([H, H], f32, tag="gcrelr")
            nc.sync.dma_start(gcrelr[:], gcrel32d[:])
            gcrelbr = pmp.tile([1, H], f32, tag="gcrelbr")
            nc.sync.dma_start(gcrelbr[:], gcrelb32d[:])
            gcrootr = pmp.tile([H, H], f32, tag="gcrootr")
            nc.sync.dma_start(gcrootr[:], gcroot32d[:])
            betawr = pmp.tile([100, 3, 3, H], f32, tag="betawr")
            nc.sync.dma_start(betawr[:], betaw32d[:].rearrange(
                "p (fc gc h) -> p fc gc h", fc=3, gc=3))
            betabr = pmp.tile([1, 3 * H], f32, tag="betabr")
            nc.sync.dma_start(betabr[:], betab32d[:])
            xTsr = pmp.tile([100, 2, NPC], f32, tag="xTsr")
            nc.sync.dma_start(xTsr[:], xTs32d[:].rearrange(
                "p (c n) -> p c n", c=2))
            Wtr = pmp.tile([100, 2, R, H], f32, tag="Wtr")
            h1Tr = pmp.tile([100, NPC], f32, tag="h1Tr")
            h2Tr = pmp.tile([100, NPC], f32, tag="h2Tr")

            # cross-phase tiles
            bef16x = pp.tile([100, 2, NPC], f16, tag="bef16x")
            bef2 = pp.tile([100, 2, NPC], f16, tag="bef2")  # h2-chunk hi/lo
            em2sb = pp.tile([128, 8, 300], f32, tag="em2sb")
            rsum = pp.tile([128, 8], f32, tag="rsum")

            # ================= phase W: relation weights (f32r) ==========
            with tc.tile_pool(name="pw", bufs=2) as pw, \
                 tc.tile_pool(name="pwc", bufs=3) as pwc, \
                 tc.tile_pool(name="psw", bufs=2, space="PSUM") as psw:
                compT = pw.tile([NB, R], f32, tag="compT")
                nc.sync.dma_start(compT[:], compT32d[:])
                CB = 2500
                for cb in range(F * H // CB):
                    bchunk = pw.tile([NB, CB], f32, tag="bchunk")
                    nc.sync.dma_start(
                        bchunk[:], basis32d[:, cb * CB:(cb + 1) * CB])
                    for t in range(CB // 500):
                        pwp = psw.tile([R, 500], f32, tag="pwp")
                        nc.tensor.matmul(pwp[:], compT[:],
                                         bchunk[:, t * 500:(t + 1) * 500],
                                         start=True, stop=True)
                        wsb = pwc.tile([R, 500], f32, tag="wsb")
                        nc.vector.tensor_copy(wsb[:], pwp[:])
                        nc.sync.dma_start(
                            wtmp_d[:, cb * CB + t * 500:
                                   cb * CB + (t + 1) * 500], wsb[:])
                # read back transposed: Wt[fp,fc,r,h] = W[r, fc*100+fp, h]
                for fc in range(2):
                    nc.sync.dma_start(
                        Wtr[:, fc, :, :],
                        wtmp_d[:, fc * 100 * H:(fc + 1) * 100 * H].rearrange(
                            "r (fp h) -> fp r h", h=H))

            # ================= stage 1: RGCN =================
            CH = 8                           # blocks per DMA chunk
            with tc.tile_pool(name="p1", bufs=3) as p1, \
                 tc.tile_pool(name="p1b", bufs=2) as p1b, \
                 tc.tile_pool(name="p1c", bufs=1) as p1c, \
                 tc.tile_pool(name="ps1", bufs=2, space="PSUM") as ps1, \
                 tc.tile_pool(name="ps1h", bufs=1, space="PSUM") as ps1h:
                aggsbr = p1c.tile([128, 2, R, NPC], f32, tag="aggsbr")
                h1T_ps = ps1h.tile([128, NPC], f32, tag="h1T_ps")

                # bin -> (group, rel); blocks laid out bin-major.
                # PSUM has_written is cleared bank-wide by start=True, so a
                # group's agg tile gets exactly ONE start (its first matmul)
                # and ONE stop (its last); fresh regions written with
                # start=False overwrite-and-set.
                binof = []
                for b_idx, nb in enumerate(B1):
                    binof += [b_idx] * nb
                gfirst = {}
                glast = {}
                for b in range(totB1):
                    gg = binof[b] >> 3
                    if gg not in gfirst:
                        gfirst[gg] = b
                    glast[gg] = b
                agg_ps = None
                gcur = -1
                bo = 0
                while bo < totB1:
                    k = min(CH, totB1 - bo)
                    xgt = p1.tile([128, CH, F], f16, tag="xgt")
                    nc.sync.dma_start(
                        xgt[:, 0:k, :],
                        xgd[:, bo * F:(bo + k) * F].rearrange(
                            "p (b f) -> p b f", f=F))
                    selt = p1.tile([128, CH, 32], f16, tag="selt")
                    nc.sync.dma_start(
                        selt[:, 0:k, :],
                        sel1d[:, bo * 32:(bo + k) * 32].rearrange(
                            "p (b d) -> p b d", d=32))
                    for j in range(k):
                        b = bo + j
                        bn = binof[b]
                        gg, rr = bn >> 3, bn & 7
                        if gg != gcur:
                            if agg_ps is not None:
                                nc.vector.tensor_copy(
                                    aggsbr[:, :, :, gcur * 32:(gcur + 1) * 32],
                                    agg_ps[:].rearrange(
                                        "p fc (r d) -> p fc r d", d=32))
                            agg_ps = ps1.tile([128, 2, R * 32], f32,
                                              tag="agg_ps")
                            gcur = gg
                        for fc in range(2):
                            nc.tensor.matmul(
                                agg_ps[0:100, fc, rr * 32:(rr + 1) * 32],
                                xgt[:, j, fc * 100:(fc + 1) * 100],
                                selt[:, j, :],
                                start=(b == gfirst[gg] and fc == 0),
                                stop=(b == glast[gg] and fc == 1))
                    bo += k
                nc.vector.tensor_copy(
                    aggsbr[:, :, :, gcur * 32:(gcur + 1) * 32],
                    agg_ps[:].rearrange("p fc (r d) -> p fc r d", d=32))

                # transform: h1T = sum_r W_r^T agg_r + root + bias (f32r)
                for hh in range(2):
                    hsl = slice(hh * 512, (hh + 1) * 512)
                    mmi = 0
                    for fc in range(2):
                        for rr in range(R):
                            nc.tensor.matmul(h1T_ps[0:100, hsl],
                                             Wtr[:, fc, rr, :],
                                             aggsbr[0:100, fc, rr, hsl],
                                             start=(mmi == 0), stop=False)
                            mmi += 1
                    for fc in range(2):
                        nc.tensor.matmul(h1T_ps[0:100, hsl], rootwr[:, fc, :],
                                         xTsr[:, fc, hsl],
                                         start=False, stop=False)
                    nc.tensor.matmul(h1T_ps[0:100, hsl], rootbr[:],
                                     onesr[:], start=False, stop=True)
                nc.vector.tensor_copy(h1Tr[:], h1T_ps[0:100, :])

                # h1 -> fp16 hi + lo pair, node-major strip, DRAM
                h1hi = p1b.tile([100, NPC], f16, tag="h1hi")
                nc.vector.tensor_copy(h1hi[:], h1T_ps[0:100, :])
                h1lo = p1b.tile([100, NPC], f16, tag="h1lo")
                nc.vector.tensor_tensor(h1lo[:], h1T_ps[0:100, :], h1hi[:],
                                        op=ALU.subtract)
                h1n = p1b.tile([128, 8, 2, H], f16, tag="h1n")
                for t in range(8):
                    for pr, part in enumerate((h1hi, h1lo)):
                        tp = ps1.tile([128, H], f16, tag="tp1")
                        nc.tensor.matmul(tp[:], part[:, t * 128:(t + 1) * 128],
                                         id16[0:100, 0:100], is_transpose=True,
                                         start=True, stop=True)
                        nc.vector.tensor_copy(h1n[:, t, pr, :], tp[:])
                nc.sync.dma_start(
                    h1s_d[:].rearrange("(b p) (pr h) -> p b pr h",
                                       p=128, pr=2), h1n[:])

            nc.gpsimd.collective_compute(
                "AllGather", ALU.bypass,
                replica_groups=[list(range(CORES))],
                ins=[h1s_d[:].opt()], outs=[h1f_d[:].opt()])

            if phase == "B":
                nc.sync.dma_start(dbgd[:], h1f_d[:])

            # ================= stage 2: GraphConv =================
            with tc.tile_pool(name="p2", bufs=3) as p2, \
                 tc.tile_pool(name="p2b", bufs=1) as p2b, \
                 tc.tile_pool(name="ps2", bufs=2, space="PSUM") as ps2, \
                 tc.tile_pool(name="ps2h", bufs=1, space="PSUM") as ps2h:
                h1blk = p2b.tile([128, KB, 2, H], f16, tag="h1blk")
                nc.sync.dma_start(
                    h1blk[:], h1f_d[:].rearrange(
                        "(kb p) (pr h) -> p kb pr h", p=128, pr=2))
                agg2_ps = ps2h.tile([128, NPC], f32, tag="agg2_ps")
                SC = 2
                for c0 in range(0, KB, SC):
                    s2t = p2.tile([128, SC, NPC], f16, tag="s2t")
                    nc.sync.dma_start(
                        s2t[:],
                        sel2d[:, c0 * NPC:(c0 + SC) * NPC].rearrange(
                            "p (b d) -> p b d", d=NPC))
                    for j in range(SC):
                        kb = c0 + j
                        for pr in range(2):
                            for hh in range(2):
                                hsl = slice(hh * 512, (hh + 1) * 512)
                                nc.tensor.matmul(
                                    agg2_ps[0:100, hsl],
                                    h1blk[:, kb, pr, :],
                                    s2t[:, j, hsl],
                                    start=(kb == 0 and pr == 0),
                                    stop=(kb == KB - 1 and pr == 1))
                agg2r = p2b.tile([100, NPC], f32, tag="agg2r")
                nc.vector.tensor_copy(agg2r[:], agg2_ps[0:100, :])
                h2T_ps = ps2h.tile([128, NPC], f32, tag="h2T_ps")
                for hh in range(2):
                    hsl = slice(hh * 512, (hh + 1) * 512)
                    nc.tensor.matmul(h2T_ps[0:100, hsl], gcrelr[:],
                                     agg2r[:, hsl], start=True, stop=False)
                    nc.tensor.matmul(h2T_ps[0:100, hsl], gcrootr[:],
                                     h1Tr[:, hsl], start=False, stop=False)
                    nc.tensor.matmul(h2T_ps[0:100, hsl], gcrelbr[:],
                                     onesr[:], start=False, stop=True)
                nc.vector.tensor_copy(h2Tr[:], h2T_ps[0:100, :])
                h2sb32 = p2b.tile([100, NPC], f32, tag="h2sb32")
                nc.vector.tensor_copy(h2sb32[:], h2T_ps[0:100, :])
                h2n = p2b.tile([128, 8, H], f32, tag="h2n")
                for t in range(8):
                    tp = ps2.tile([128, H], f32, tag="tp2")
                    nc.tensor.matmul(tp[:], h2sb32[:, t * 128:(t + 1) * 128],
                                     id32[0:100, 0:100], is_transpose=True,
                                     start=True, stop=True)
                    nc.vector.tensor_copy(h2n[:, t, :], tp[:])
                nc.sync.dma_start(
                    h2s_d[:].rearrange("(b p) h -> p b h", p=128), h2n[:])

            nc.gpsimd.collective_compute(
                "AllGather", ALU.bypass,
                replica_groups=[list(range(CORES))],
                ins=[h2s_d[:].opt()], outs=[h2f_d[:].opt()])

            # ---- before = emotions @ beta_w + b (own strip; overlaps AG2)
            with tc.tile_pool(name="psb", bufs=2, space="PSUM") as psb:
                emoTs = (xTsr[:, 0, :], xTsr[:, 1, :], h2Tr[:])
                for gc in range(3):
                    bps = psb.tile([100, NPC], f32, tag="bps")
                    for hh in range(2):
                        hsl = slice(hh * 512, (hh + 1) * 512)
                        for fc in range(3):
                            nc.tensor.matmul(bps[:, hsl],
                                             betawr[:, fc, gc, :],
                                             emoTs[fc][:, hsl],
                                             start=(fc == 0), stop=False)
                        nc.tensor.matmul(bps[:, hsl],
                                         betabr[:, gc * H:(gc + 1) * H],
                                         onesr[:], start=False, stop=True)
                    if gc < 2:
                        nc.vector.tensor_copy(bef16x[:, gc, :], bps[:])
                    else:
                        nc.vector.tensor_copy(bef2[:, 0, :], bps[:])
                        nc.vector.tensor_tensor(bef2[:, 1, :], bps[:],
                                                bef2[:, 0, :],
                                                op=ALU.subtract)
            pm.__exit__(None, None, None)

            # ================= keys + V + attention + head ===============
            with tc.tile_pool(name="pk", bufs=1) as pk:
                keysx = pk.tile([100, 2, N], f16, tag="keysx")
                nc.sync.dma_start(keysx[:, 0, :], xT16d[0:100, :])
                nc.sync.dma_start(keysx[:, 1, :], xT16d[100:200, :])
                emoV = pk.tile([128, KB, 300], f16, tag="emoV")
                nc.sync.dma_start(
                    emoV[:, :, 0:F], xvd[:].rearrange("p (kb f) -> p kb f",
                                                      f=F))
                keys2 = pk.tile([100, 2, N], f16, tag="keys2")  # hi/lo
                # h2 keys: f32 transposes of the AllGathered h2
                with tc.tile_pool(name="pkc", bufs=2) as pkc, \
                     tc.tile_pool(name="psk", bufs=2, space="PSUM") as psk:
                    for t0 in range(0, KB, 8):
                        h2fb = pkc.tile([128, 8, H], f32, tag="h2fb")
                        nc.sync.dma_start(
                            h2fb[:],
                            h2f_d[t0 * 128:(t0 + 8) * 128, :].rearrange(
                                "(kb p) h -> p kb h", p=128))
                        nc.vector.tensor_copy(emoV[:, t0:t0 + 8, F:300],
                                              h2fb[:])
                        for t in range(8):
                            tp = psk.tile([100, 128], f32, tag="tpk")
                            nc.tensor.matmul(tp[:], h2fb[:, t, :], id32[:],
                                             is_transpose=True,
                                             start=True, stop=True)
                            ks = slice((t0 + t) * 128, (t0 + t + 1) * 128)
                            nc.vector.tensor_copy(keys2[:, 0, ks], tp[:])
                            nc.vector.tensor_tensor(keys2[:, 1, ks], tp[:],
                                                    keys2[:, 0, ks],
                                                    op=ALU.subtract)

                if phase == "C":
                    nc.sync.dma_start(dbgd[:, 0:H],
                                      h1f_d[:].rearrange(
                                          "n (pr h) -> n pr h", pr=2)[:, 0, :])

                # ---- attention: q-tiles of 128, true row-max softmax
                with tc.tile_pool(name="pq", bufs=1) as pq, \
                     tc.tile_pool(name="pq2", bufs=2) as pq2, \
                     tc.tile_pool(name="pl", bufs=3) as pl, \
                     tc.tile_pool(name="pse", bufs=2, space="PSUM") as pse, \
                     tc.tile_pool(name="psp", bufs=2, space="PSUM") as psp, \
                     tc.tile_pool(name="pss", bufs=3, space="PSUM") as pss:
                    for qt in range(8):
                        qsl = slice(qt * 128, (qt + 1) * 128)
                        srow = pq.tile([128, N], f32, tag="srow")
                        mxc = pl.tile([128, 16], f32, tag="mxc")
                        for kt in range(16):
                            ksl = slice(kt * 512, (kt + 1) * 512)
                            sps = pss.tile([128, 512], f32, tag="sps")
                            for fc in range(2):
                                nc.tensor.matmul(sps[:], bef16x[:, fc, qsl],
                                                 keysx[:, fc, ksl],
                                                 start=(fc == 0), stop=False)
                            nc.tensor.matmul(sps[:], bef2[:, 0, qsl],
                                             keys2[:, 0, ksl],
                                             start=False, stop=False)
                            nc.tensor.matmul(sps[:], bef2[:, 0, qsl],
                                             keys2[:, 1, ksl],
                                             start=False, stop=False)
                            nc.tensor.matmul(sps[:], bef2[:, 1, qsl],
                                             keys2[:, 0, ksl],
                                             start=False, stop=True)
                            nc.vector.tensor_copy(srow[:, ksl], sps[:])
                            nc.vector.reduce_max(mxc[:, kt:kt + 1], sps[:],
                                                 axis=AX.XYZW)
                        mx = pl.tile([128, 1], f32, tag="mx")
                        nc.vector.reduce_max(mx[:], mxc[:], axis=AX.XYZW)
                        nmx = pl.tile([128, 1], f32, tag="nmx")
                        nc.vector.tensor_scalar_mul(nmx[:], mx[:], -1.0)
                        prow = pq2.tile([128, N], f16, tag="prow")
                        nc.scalar.activation(prow[:], srow[:], AF.Exp,
                                             bias=nmx[:], scale=1.0,
                                             accum_out=rsum[:, qt:qt + 1])
                        em2_ps = pse.tile([128, 300], f32, tag="em2_ps")
                        for kb in range(KB):
                            ptp = psp.tile([128, 128], f16, tag="ptp")
                            nc.tensor.matmul(ptp[:],
                                             prow[:, kb * 128:(kb + 1) * 128],
                                             id16[:], is_transpose=True,
                                             start=True, stop=True)
                            pts = pl.tile([128, 128], f16, tag="pts")
                            nc.vector.tensor_copy(pts[:], ptp[:])
                            nc.tensor.matmul(em2_ps[:], pts[:],
                                             emoV[:, kb, :],
                                             start=(kb == 0),
                                             stop=(kb == KB - 1))
                        nc.vector.tensor_copy(em2sb[:, qt, :], em2_ps[:])

            # ================= head =================
            with tc.tile_pool(name="ph", bufs=2) as ph, \
                 tc.tile_pool(name="psh", bufs=1, space="PSUM") as psh:
                for qt in range(8):
                    rcp = ph.tile([128, 1], f32, tag="rcp")
                    nc.vector.reciprocal(rcp[:], rsum[:, qt:qt + 1])
                    em2n = ph.tile([128, 3, H], f16, tag="em2n")
                    nc.vector.tensor_scalar(em2n[:],
                                            em2sb[:, qt, :].rearrange(
                                                "p (c h) -> p c h", c=3),
                                            rcp[:], None, op0=ALU.mult)
                    e2t = ph.tile([H, 3, 128], f16, tag="e2t")
                    for c in range(3):
                        tp = psh.tile([H, 128], f16, tag="tpb")
                        nc.tensor.matmul(tp[:], em2n[:, c, :],
                                         id16[:], is_transpose=True,
                                         start=True, stop=True)
                        nc.vector.tensor_copy(e2t[:, c, :], tp[:])
                    hid_ps = psh.tile([H, 128], f32, tag="hid_ps")
                    for c in range(3):
                        nc.tensor.matmul(hid_ps[:], linw[:, c, :],
                                         e2t[:, c, :],
                                         start=(c == 0), stop=False)
                    nc.tensor.matmul(hid_ps[:], linb[:], ones16[:, 0:128],
                                     start=False, stop=True)
                    hidT = ph.tile([H, 128], f16, tag="hidT")
                    nc.scalar.activation(hidT[:], hid_ps[:], AF.Relu)
                    lg_ps = psh.tile([NC, 128], f32, tag="lg_ps")
                    nc.tensor.matmul(lg_ps[:], smaxw[:], hidT[:],
                                     start=True, stop=False)
                    nc.tensor.matmul(lg_ps[:], smaxb[:], ones16[:, 0:128],
                                     start=False, stop=True)
                    lgT = ph.tile([NC, 128], f32, tag="lgT")
                    nc.vector.tensor_copy(lgT[:], lg_ps[:])
                    plt = psh.tile([128, NC], f32, tag="plt")
                    nc.tensor.matmul(plt[:], lgT[:], id32[0:NC, 0:NC],
                                     is_transpose=True, start=True, stop=True)
                    lg = ph.tile([128, NC], f32, tag="lg")
                    nc.vector.tensor_copy(lg[:], plt[:])
                    m6 = ph.tile([128, 1], f32, tag="m6")
                    nc.vector.reduce_max(m6[:], lg[:], axis=AX.XYZW)
                    nm6 = ph.tile([128, 1], f32, tag="nm6")
                    nc.vector.tensor_scalar_mul(nm6[:], m6[:], -1.0)
                    e6 = ph.tile([128, NC], f32, tag="e6")
                    s6 = ph.tile([128, 1], f32, tag="s6")
                    nc.scalar.activation(e6[:], lg[:], AF.Exp, bias=nm6[:],
                                         scale=1.0, accum_out=s6[:])
                    ls6 = ph.tile([128, 1], f32, tag="ls6")
                    nc.scalar.activation(ls6[:], s6[:], AF.Ln)
                    sh = ph.tile([128, 1], f32, tag="sh")
                    nc.vector.tensor_add(sh[:], m6[:], ls6[:])
                    outt = ph.tile([128, NC], f32, tag="outt")
                    nc.vector.tensor_scalar(outt[:], lg[:], sh[:], None,
                                            op0=ALU.subtract)
                    nc.sync.dma_start(outd[qt * 128:(qt + 1) * 128, :],
                                      outt[:])

    nc.compile()
    return nc


# ------------------------------------------------------------------ entry
def kernel(x, edge_index, edge_norm, edge_type, basis, comp, root_w, root_b,
           gc_rel_w, gc_rel_b, gc_root_w, beta_w, beta_b, lin_w, lin_b,
           smax_w, smax_b):
    x = np.ascontiguousarray(np.asarray(x, np.float32))
    per_core, meta = _prep(x, edge_index, edge_type)

    import os
    phase = os.environ.get("KPHASE", "full")
    key = (phase, meta["totB1"], tuple(meta["B1"]))
    if key not in _ker_cache:
        _ker_cache[key] = _build(meta, phase)
    nc = _ker_cache[key]

    x16 = x.astype(np.float16)
    xT = np.ascontiguousarray(x.T)
    linw_pack = np.ascontiguousarray(
        np.asarray(lin_w, np.float16).reshape(3, 100, H).transpose(
            1, 0, 2)).reshape(100, 3 * H)
    bw = np.asarray(beta_w, np.float32)           # [300, 300]
    betawT = np.ascontiguousarray(
        bw.reshape(3, 100, 3, 100).transpose(1, 0, 2, 3)).reshape(100, -1)
    rootw32 = np.ascontiguousarray(
        np.asarray(root_w, np.float32).reshape(2, 100, H).transpose(
            1, 0, 2)).reshape(100, 2 * H)

    shared = dict(
        xT16=np.ascontiguousarray(x16.T),
        xv=np.ascontiguousarray(
            x16.reshape(KB, 128, F).transpose(1, 0, 2)).reshape(128, KB * F),
        basis32=np.ascontiguousarray(
            np.asarray(basis, np.float32).reshape(NB, F * H)),
        compT32=np.ascontiguousarray(np.asarray(comp, np.float32).T),
        rootw32=rootw32,
        rootb32=np.asarray(root_b, np.float32).reshape(1, H),
        gcrel32=np.asarray(gc_rel_w, np.float32),
        gcrelb32=np.asarray(gc_rel_b, np.float32).reshape(1, H),
        gcroot32=np.asarray(gc_root_w, np.float32),
        betaw32=betawT,
        betab32=np.asarray(beta_b, np.float32).reshape(1, 3 * H),
        linw16=linw_pack,
        linb16=np.asarray(lin_b, np.float16).reshape(1, H),
        smaxw16=np.asarray(smax_w, np.float16),
        smaxb16=np.asarray(smax_b, np.float16).reshape(1, NC),
        ident16=np.eye(128, dtype=np.float16),
        ident32=np.eye(128, dtype=np.float32),
        ones32=np.ones((1, 512), np.float32),
        ones16=np.ones((1, NPC), np.float16),
    )
    in_maps = []
    for c in range(CORES):
        m = dict(shared)
        strip = xT[:, c * NPC:(c + 1) * NPC]
        m["xTs32"] = np.ascontiguousarray(
            strip.reshape(2, 100, NPC).transpose(1, 0, 2)).reshape(
            100, 2 * NPC)
        m.update(per_core[c])
        in_maps.append(m)

    res = run_bass_kernel_spmd(nc, in_maps, core_ids=list(range(CORES)),
                               trace_cores=[0])
    global _last_res
    _last_res = res
    if phase != "full":
        return [res.results[c]["dbg"] for c in range(CORES)]
    return np.concatenate([res.results[c]["out"] for c in range(CORES)],
                          axis=0)
